# revision 27
# baseline (speedup 1.0000x reference)
"""GCN feature extractor on 8 Trainium2 NeuronCores.

Row-parallel sharding over the dense normalized adjacency A (symmetric).
Each core c owns a 1024-node block and computes, entirely on-device:

  Y    = X^T @ A[:, blk]                    ([FIN, BLK])   K=N matmul
  M    = Ppool @ A[:, blk]                  ([B, BLK])     same stream!
  H1^T = relu(W1^T @ Y + b1 (x) rowsums)    ([HID, BLK])
  Z    = H1 @ W2 + b2                       ([BLK, OUT])
  out  = Z^T @ M^T                          ([OUT, B])     partial

The host sums the 8 [OUT, B] partials (the pooling "all-reduce") and
transposes. The K=8192 adjacency contractions (Y and M) run as fp8e4
DoubleRow matmuls: X, Ppool and C all quantize to e4m3 (C entries
{0,1,2} are exact; X/Ppool cost ~1e-2 rel err, inside the 2e-2 gate),
the PE packs two K-rows per cell, and phase 2a drops from 131072 to
65536 moving columns. The fused stream is 32 super-chunks of
[128, 2, 1216] u8 (two 128-row K-halves: [X8 | C | Ppool8] each),
alternating between the two HWDGE rings (sync/scalar) for bandwidth.
1/count folds into the final cinv multiply; b1's adjacency product is
a zero-padded rank-1 matmul with host-precomputed rowsums(A).
"""

import numpy as np
import ml_dtypes

import concourse.bass as bass
import concourse.mybir as mybir
import concourse.tile as tile
from concourse.masks import make_identity
from concourse.vector_clock import ScopedClock
from concourse.bass_utils import run_bass_kernel_spmd

N, FIN, HID, OUT, B, NCORES = 8192, 128, 256, 128, 64, 8
BLK = N // NCORES  # 1024
P = 128
SC = 32            # K super-chunks of 256 rows (2 halves of 128)
HB = FIN + BLK + 2 * B  # 1280 bytes per half: X8 | C | Ppool bf16

DT = mybir.dt.bfloat16
F8 = mybir.dt.float8e4
NP_DT = ml_dtypes.bfloat16
NP_F8 = ml_dtypes.float8_e4m3
DR = mybir.MatmulPerfMode.DoubleRow
F32R = mybir.dt.float32r


def _legalize_waits(nc, max_waits=1):
    """This walrus build only accepts a single semaphore wait per
    instruction; Tile attaches as many as the dependence structure
    needs. Hoist excess waits onto pure-wait EventSemaphore
    instructions (what wait_ge emits) inserted just before the owner."""

    def fix_block(blk):
        for sub in getattr(blk, "blocks", None) or []:
            fix_block(sub)
        insts = list(blk.instructions)
        out = []
        changed = False
        for inst in insts:
            si = getattr(inst, "sync_info", None)
            waits = list(si.on_wait) if si is not None else []
            if len(waits) > max_waits:
                changed = True
                inst.sync_info = mybir.SyncInfo(
                    on_wait=waits[-max_waits:], on_update=list(si.on_update)
                )
                for j, w in enumerate(waits[:-max_waits]):
                    out.append(
                        mybir.InstEventSemaphore(
                            name=f"{inst.name}-hw{j}",
                            engine=inst.engine,
                            ins=[],
                            outs=[],
                            sync_info=mybir.SyncInfo(on_wait=[w], on_update=[]),
                        )
                    )
            out.append(inst)
        if changed:
            blk.instructions = out

    for fn in nc.m.functions:
        for blk in fn.blocks:
            fix_block(blk)


class _TileContext(tile.TileContext):
    def _drain_and_barrier(self, tick_clock, wait_clock):
        nc = self.nc
        drain_inst = nc.sync.drain()
        wait_clock.add_sem_waits(
            drain_inst.ins, ScopedClock({None: tick_clock.global_clock})
        )
        si = drain_inst.ins.sync_info
        waits = list(si.on_wait) if si is not None else []
        if len(waits) > 1:
            drain_inst.ins.sync_info = mybir.SyncInfo(
                on_wait=waits[:1], on_update=list(si.on_update)
            )
            for w in waits[1:]:
                extra = nc.sync.drain()
                extra.ins.sync_info = mybir.SyncInfo(on_wait=[w], on_update=[])
        nc.all_engine_barrier()
        popped = nc._tile_sem_poison_stack.pop()
        assert popped is self._sem_poison
        assert self.sems is not None
        nc.clear_and_free_semaphores(list(self.sems.allocated().values()))
        nc.all_engine_barrier()


def build_program(detect_races=True):
    # enable_partition_id=False: the SPMD preamble's partition-id DMA +
    # barrier costs ~3us of all-engine wait at t=0; per-core inputs are
    # already distinct so the id is unused.
    nc = bass.Bass(
        detect_race_conditions=detect_races, enable_partition_id=False
    )
    f32 = mybir.dt.float32

    axk_d = nc.dram_tensor("axk", [SC, P, 2, HB], mybir.dt.uint8,
                           kind="ExternalInput")
    dinvy_d = nc.dram_tensor("dinvy", [P, BLK], DT, kind="ExternalInput")
    dinvm_d = nc.dram_tensor("dinvm", [P, 512], DT, kind="ExternalInput")
    w1_d = nc.dram_tensor("w1", [P, HID], DT, kind="ExternalInput")
    # b1row/rrow: row 0 carries b1 / rowsums(A)_blk, rows 1..127 zero —
    # the b1 (x) r rank-1 update runs as a full K=128 matmul (K<128
    # matmuls are broken in this stack).
    b1row_d = nc.dram_tensor("b1row", [P, HID], DT, kind="ExternalInput")
    rrow_d = nc.dram_tensor("rrow", [P, BLK], DT, kind="ExternalInput")
    w2_d = nc.dram_tensor("w2", [2, P, OUT], DT, kind="ExternalInput")
    b2r_d = nc.dram_tensor("b2r", [P, OUT], f32, kind="ExternalInput")
    cinv_d = nc.dram_tensor("cinv", [P, B], f32, kind="ExternalInput")
    ident_d = nc.dram_tensor("ident", [P, P], f32, kind="ExternalInput")
    out_d = nc.dram_tensor("outp", [P, B], f32, kind="ExternalOutput")

    with _TileContext(nc) as tc:
        with (
            tc.tile_pool(name="const", bufs=1) as cpool,
            tc.tile_pool(name="sb", bufs=1) as spool,
            tc.tile_pool(name="acol", bufs=24) as apool,
            tc.tile_pool(name="ps", bufs=1, space="PSUM") as pspool,
        ):
            # --- SBUF tiles ---
            w1_sb = cpool.tile([P, HID], DT)
            b1row_sb = cpool.tile([P, HID], DT)
            rrow_sb = cpool.tile([P, BLK], DT)
            w2_sb = [
                cpool.tile([P, OUT], DT, tag=f"w2_{k}", name=f"w2_{k}")
                for k in range(2)
            ]
            b2r_sb = cpool.tile([P, OUT], f32)
            cinv_sb = cpool.tile([P, B], f32)
            dinvy_sb = cpool.tile([P, BLK], DT)
            dinvm_sb = cpool.tile([P, 512], DT)
            ident_sb = cpool.tile([P, P], f32)
            warm_sb = cpool.tile([P, 8], f32)

            y_sb = spool.tile([P, BLK], DT)
            m_sb = spool.tile([P, 512], f32)
            h1t_sb = [
                spool.tile([P, BLK], DT, tag=f"h1t_{m}", name=f"h1t_{m}")
                for m in range(2)
            ]
            z_sb = [
                spool.tile([P, OUT], DT, tag=f"z_{m}", name=f"z_{m}")
                for m in range(8)
            ]
            # M^T packed: col c*128+0:64 = nodes c*128.. (graphs), col
            # c*128+64:128 = nodes 512+c*128.. — one copy fills all 8.
            mtall = spool.tile([P, 512], DT)
            osb = spool.tile([P, B], f32)

            # --- PSUM: 8 banks, one scope, no drains. psy/psm live in a
            # 3-buf cycling pool whose banks the 8 psz tiles later reuse
            # (distinct tile objects so Z matmuls/bias-adds pipeline —
            # dep tracking is tile-granular). ---
            psy = [
                pspool.tile([P, 512], f32, tag="zz", name=f"psy_{nn}",
                            bufs=3)
                for nn in range(2)
            ]
            psm = pspool.tile([P, 512], f32, tag="zz", name="psm", bufs=3)
            psh = [
                [
                    pspool.tile([P, 512], f32, tag=f"psh_{mc}_{nn}",
                                name=f"psh_{mc}_{nn}")
                    for nn in range(2)
                ]
                for mc in range(2)
            ]
            ptail = pspool.tile([P, 512], f32, tag="ptail", name="ptail")




            # --- Phase 2a: Y = X^T C and M = Ppool^T C over one fused
            # fp8 DoubleRow stream; K = 256 per super-chunk. ---
            for s in range(SC):
                ac = apool.tile([P, 2, HB], mybir.dt.uint8, tag="ac",
                                name=f"ac_{s}")
                if s == 0:
                    # split chunk 0 across both rings: halves the time
                    # to the first matmul's data dependency.
                    nc.sync.dma_start(ac[:, 0, :], axk_d[0, :, 0])
                    nc.scalar.dma_start(ac[:, 1, :], axk_d[0, :, 1])
                else:
                    # chunk 1 on sync so the PE can start without the
                    # scalar ring; then alternate rings.
                    eng = nc.sync if (s < 2 or s % 2 == 0) else nc.scalar
                    eng.dma_start(ac[:], axk_d[s])
                if s == 3:
                    # preload the Relu activation table while the PE is
                    # still cold; the real relus at ~30us skip the load.
                    nc.scalar.activation(
                        warm_sb[:], ident_sb[:, 0:8],
                        mybir.ActivationFunctionType.Relu,
                    )
                xs = ac[:, :, 0:FIN].bitcast(F8)
                cs = [
                    ac[:, :, FIN + nn * 512 : FIN + (nn + 1) * 512].bitcast(F8)
                    for nn in range(2)
                ]
                st, sp = (s == 0), (s == SC - 1)
                # Keep the two DoubleRow Y matmuls adjacent: a DR matmul
                # holds both PE weight buffers, so alternating DR and
                # normal-mode matmuls serializes every LDWEIGHTS
                # (measured +19us). Pool runs normal-mode (DoubleRow
                # can't write to a partition-offset dst): per K-half,
                # two col-tiled MMs into the stacked psm.
                # skip_group_check: the sim's zero-region tracker
                # ignores partition bases and miscounts the two
                # partition-split groups sharing this bank; HW
                # has_written is per-element (baseline-proven).
                for nn in range(2):
                    nc.tensor.matmul(
                        psy[nn][:], xs, cs[nn], start=st, stop=sp,
                        perf_mode=DR,
                    )
                for j in range(2):
                    ppj = ac[:, j, FIN + BLK : HB].bitcast(DT)
                    for kk in range(2):
                        nc.tensor.matmul(
                            psm[kk * B : (kk + 1) * B, :],
                            ppj,
                            ac[:, j, FIN + kk * 512 : FIN + (kk + 1) * 512]
                            .bitcast(F8),
                            start=(st and j == 0), stop=(sp and j == 1),
                            tile_position=(0, kk * B),
                            skip_group_check=True,
                        )

            # consts land on the scalar ring after its chunk stream;
            # all are first consumed at phase-2a end (~30us).
            nc.scalar.dma_start(w1_sb[:], w1_d[:])
            nc.scalar.dma_start(b1row_sb[:], b1row_d[:])
            nc.scalar.dma_start(rrow_sb[:], rrow_d[:])
            for k in range(2):
                nc.scalar.dma_start(w2_sb[k][:], w2_d[k])
            nc.scalar.dma_start(b2r_sb[:], b2r_d[:])
            nc.scalar.dma_start(cinv_sb[:], cinv_d[:])
            nc.scalar.dma_start(dinvy_sb[:], dinvy_d[:])
            nc.scalar.dma_start(dinvm_sb[:], dinvm_d[:])
            # identity for the PE transposes comes from the host (a
            # make_identity on gpsimd materializes framework const
            # tensors in the preamble). The warm activation above reads
            # ident_sb before this DMA lands — garbage is fine there,
            # only the table load matters.
            nc.scalar.dma_start(ident_sb[:], ident_d[:])

            # --- Tail. PE order: b1 rank-1 MMs (no Y dep) -> transposes
            # (after m-scale) -> W1 MMs (after y-scale) -> Z -> out. ---
            for mc in range(2):
                for nn in range(2):
                    nc.tensor.matmul(
                        psh[mc][nn][:],
                        b1row_sb[:, mc * P : (mc + 1) * P],
                        rrow_sb[:, nn * 512 : (nn + 1) * 512],
                        start=True, stop=False,
                    )

            # DVE: m-scale first (unblocks transposes), then y-scales.
            nc.vector.tensor_tensor(
                m_sb[:], psm[:], dinvm_sb[:], mybir.AluOpType.mult
            )
            for nn in range(2):
                nc.vector.tensor_tensor(
                    y_sb[:, nn * 512 : (nn + 1) * 512],
                    psy[nn][:],
                    dinvy_sb[:, nn * 512 : (nn + 1) * 512],
                    mybir.AluOpType.mult,
                )

            # M^T chunks via PE transpose into ptail slots, then one
            # wide copy to SBUF.
            for c in range(4):
                nc.tensor.transpose(
                    ptail[:, c * P : (c + 1) * P],
                    m_sb[:, c * P : (c + 1) * P], ident_sb[:]
                )
            nc.vector.tensor_copy(mtall[:], ptail[:])

            for mc in range(2):
                for nn in range(2):
                    nc.tensor.matmul(
                        psh[mc][nn][:],
                        w1_sb[:, mc * P : (mc + 1) * P],
                        y_sb[:, nn * 512 : (nn + 1) * 512],
                        start=False, stop=True,
                    )
            # relus split across Scalar/Vector to run in parallel.
            for mc in range(2):
                for nn in range(2):
                    dst = h1t_sb[mc][:, nn * 512 : (nn + 1) * 512]
                    if nn == 0:
                        nc.scalar.activation(
                            dst, psh[mc][nn][:],
                            mybir.ActivationFunctionType.Relu,
                        )
                    else:
                        nc.vector.tensor_scalar_max(dst, psh[mc][nn][:], 0.0)

            # Phase 3: Z = H1 @ W2 + b2 (nodes-on-partitions); psz tiles
            # cycle the 3 freed phase-2a banks so MMs and bias-adds
            # pipeline.
            for mz in range(8):
                psz = pspool.tile([P, 512], f32, tag="zz", name=f"psz_{mz}",
                                  bufs=3)
                slot = psz[:, 0:OUT]
                for kz in range(2):
                    nc.tensor.matmul(
                        slot,
                        h1t_sb[kz][:, mz * P : (mz + 1) * P],
                        w2_sb[kz][:],
                        start=(kz == 0), stop=(kz == 1),
                    )
                nc.vector.tensor_tensor(
                    z_sb[mz][:], slot, b2r_sb[:], mybir.AluOpType.add
                )

            # Output: pooled^T partial = Z^T @ M^T ([OUT, B], bf16 1-pass)
            pso = ptail[:, 0:B]
            for kz in range(8):
                mt = (
                    mtall[:, kz * P : kz * P + B]
                    if kz < 4
                    else mtall[:, (kz - 4) * P + B : (kz - 3) * P]
                )
                nc.tensor.matmul(
                    pso,
                    z_sb[kz][:],
                    mt,
                    start=(kz == 0), stop=(kz == 7),
                )
            nc.vector.tensor_tensor(
                osb[:], pso, cinv_sb[:], mybir.AluOpType.mult
            )
            nc.sync.dma_start(out_d[:], osb[:])

    _legalize_waits(nc)
    return nc


def _host_prep(node_features, W1, b1, W2, b2, edge_index, batch, num_graphs):
    x = np.asarray(node_features, dtype=np.float32)
    W1 = np.asarray(W1, dtype=np.float32)
    b1 = np.asarray(b1, dtype=np.float32)
    W2 = np.asarray(W2, dtype=np.float32)
    b2 = np.asarray(b2, dtype=np.float32)
    ei = np.asarray(edge_index).astype(np.int64)
    batch = np.asarray(batch).astype(np.int64)
    nb = int(num_graphs)

    n = x.shape[0]
    # The reference's normalized adjacency factors as D @ C @ D with
    # C = (symmetrized 0/1 adjacency, dedup) + I (so a self-edge gives
    # 2.0) and D = diag(1/sqrt(deg)). C's entries {0,1,2} are exact in
    # fp8; the D scales fold into the streamed X/Ppool (left) and the
    # on-device dinv multiplies (right).
    C = np.zeros((n, n), dtype=np.uint8)
    C[ei[0], ei[1]] = 1
    C[ei[1], ei[0]] = 1
    C[np.arange(n), np.arange(n)] += 1
    deg = C.sum(axis=1, dtype=np.int64).astype(np.float32)
    dis = np.where(deg > 0, 1.0 / np.sqrt(deg, dtype=np.float32), 0.0).astype(
        np.float32
    )
    # rowsums of the normalized adjacency: dinv * (C @ dinv)
    rs = dis * (C.astype(np.float32) @ dis)

    counts = np.bincount(batch, minlength=nb).astype(np.int64)
    cinv = (1.0 / np.maximum(counts, 1)).astype(np.float32)
    cinvr = np.broadcast_to(cinv, (P, B)).copy()

    # dinv-scaled pool matrix (D @ Ppool); 1/count applies on-device.
    ppool = np.zeros((n, B), dtype=np.float32)
    ppool[np.arange(n), batch] = dis[np.arange(n)]

    w1t = W1.astype(NP_DT)  # [FIN, HID]
    b1pad = np.zeros((P, HID), dtype=np.float32)
    b1pad[0] = b1
    w2t = W2.reshape(2, P, OUT).astype(NP_DT)
    b2r = np.broadcast_to(b2, (P, OUT)).copy()

    dxu = (dis[:, None] * x).astype(NP_F8).view(np.uint8)   # [N, FIN]
    pru = ppool.astype(NP_DT).view(np.uint8)                # [N, 2B]

    in_maps = []
    for c in range(NCORES):
        lo, hi = c * BLK, (c + 1) * BLK
        rpad = np.zeros((P, BLK), dtype=np.float32)
        rpad[0] = rs[lo:hi]
        cu = np.ascontiguousarray(C[:, lo:hi]).astype(NP_F8).view(np.uint8)
        arr = np.concatenate([dxu, cu, pru], axis=1)        # [N, HB]
        axk = np.ascontiguousarray(
            arr.reshape(SC, 2, P, HB).transpose(0, 2, 1, 3)
        )
        dinvy = np.broadcast_to(dis[lo:hi], (P, BLK)).astype(NP_DT).copy()
        dinvm = np.concatenate(
            [
                np.broadcast_to(dis[lo : lo + 512], (B, 512)),
                np.broadcast_to(dis[lo + 512 : hi], (B, 512)),
            ],
            axis=0,
        ).astype(NP_DT).copy()
        in_maps.append(
            {
                "axk": axk,
                "w1": w1t,
                "b1row": b1pad.astype(NP_DT),
                "rrow": rpad.astype(NP_DT),
                "w2": w2t,
                "b2r": b2r,
                "cinv": cinvr,
                "dinvy": dinvy,
                "dinvm": dinvm,
                "ident": np.eye(P, dtype=np.float32),
            }
        )
    return in_maps, [], nb


def kernel(
    node_features, W1, b1, W2, b2, edge_index, batch, num_graphs, **_unused
):
    in_maps, _, nb = _host_prep(
        node_features, W1, b1, W2, b2, edge_index, batch, num_graphs
    )
    nc = build_program()
    try:
        res = run_bass_kernel_spmd(nc, in_maps, core_ids=list(range(NCORES)))
    except Exception:
        # Transient NRT exec-unit wedges recover on retry.
        res = run_bass_kernel_spmd(nc, in_maps, core_ids=list(range(NCORES)))
    acc = np.zeros((P, B), dtype=np.float32)
    for r in res.results:
        acc += r["outp"]
    return np.ascontiguousarray(acc.T[:nb]).astype(np.float32)
